# revision 42
# baseline (speedup 1.0000x reference)
"""Trainium2 Bass kernel for the MHA-with-diagonal-softmax module.

Computation (per batch b):
    q = rope(x @ Wq.T), k = rope(x @ Wk.T), v = x @ Wv.T      (per head, DH=128)
    sumexp[s,h] = sum_k exp(q_h[s] . k_h[k] * DH^-0.5)
    diag[s,h]   = q_h[s] . k_h[s] * DH^-0.5
    w = exp(diag) / sumexp
    out = (w * v) @ Wo.T

Sharding: 8 cores = 2 (batch) x 4 (head groups of 4 heads).
Each core computes q/k/v for its 4 heads in transposed [head_dim, seq]
layout, the per-position softmax-diagonal weights, and a partial output
projection (its heads' rows of Wo), written as 2 head-pair partials that
the host sums.

Schedule (v2): the exp(scores) stream on the ACT engine is the second-
largest engine load (~180us) and is started as early as possible (~32us,
right after head 0's K/Q projections + rope). All remaining PE work
(K/Q proj heads 1-3, V proj, output proj of pair 0) is emitted as
fine-grained filler between score matmuls so the PE queue never stalls
(in-order queues: a waiting instruction blocks everything behind it) and
the PE stays at the 2.4GHz p-state. Scores PSUM is a rotating 3-buffer
of [128,1024] halves (6 banks) + a 2-buffer [128,512] pool (2 banks) for
everything else, so score matmuls never wait on exp drain.

On-chip dtype is fp16 (same PE throughput as bf16, 8x lower rounding
error - matters because exp() amplifies absolute score error), with fp32
PSUM accumulation everywhere.
"""

import numpy as np
from contextlib import ExitStack

# Problem constants (hardcoded per harness contract).
B, S, D, H, DH = 2, 2048, 2048, 16, 128
HPC = 4            # heads per core
NHL = HPC * DH     # 512 local head dims per core
KB = D // 128      # 16 contraction blocks
SB = S // 128      # 16 seq blocks of 128
SC = S // 512      # 4 seq/emb chunks of 512
NCORES = 8

_CACHE = {}


def _build_nc():
    import concourse.bass as bass
    import concourse.tile as tile
    from concourse import bacc, mybir
    from concourse.masks import make_identity

    F16 = mybir.dt.float16
    F32 = mybir.dt.float32
    F8 = mybir.dt.float8e4
    AF = mybir.ActivationFunctionType
    DR = mybir.MatmulPerfMode.DoubleRow

    # Bacc (not raw Bass): its compile() splits multi-sem waits into
    # event-semaphore instructions - HW allows at most 1 wait per inst.
    nc = bacc.Bacc("TRN2", target_bir_lowering=False, debug=False)

    # weights arrive pre-arranged partition-major on the host so each DMA
    # is 128 x 8KB contiguous descriptors (1KB-row descriptors measured
    # ~120GB/s and hog the 4-deep DMA rings)
    xT = nc.dram_tensor("xT", [D, S], F16, kind="ExternalInput").ap()
    wq = nc.dram_tensor("wq", [128, KB * 512], F16, kind="ExternalInput").ap()
    wk = nc.dram_tensor("wk", [128, KB * 512], F16, kind="ExternalInput").ap()
    wv = nc.dram_tensor("wv", [128, KB * 512], F16, kind="ExternalInput").ap()
    wo = nc.dram_tensor("wo", [128, HPC * S], F16, kind="ExternalInput").ap()
    ropeA = nc.dram_tensor("ropeA", [128, S], F16, kind="ExternalInput").ap()
    ropeB = nc.dram_tensor("ropeB", [128, S], F16, kind="ExternalInput").ap()
    y = nc.dram_tensor("y", [2, S, D], F16, kind="ExternalOutput").ap()

    xT_r = xT.rearrange("(a p) s -> a p s", p=128)
    wq_p = wq.rearrange("p (a m) -> p a m", a=KB)
    wk_p = wk.rearrange("p (a m) -> p a m", a=KB)
    wv_p = wv.rearrange("p (a m) -> p a m", a=KB)
    wo_p = wo.rearrange("p (h n) -> p h n", h=HPC)

    with tile.TileContext(nc) as tc, ExitStack() as ctx:
        pool = ctx.enter_context(tc.tile_pool(name="sb", bufs=1))
        pp = ctx.enter_context(tc.tile_pool(name="ps", bufs=1, space="PSUM"))

        # ---- constants (gpsimd SWDGE: small, keeps HWDGE queues for x/w) --
        ra = pool.tile([128, S], F16, name="ra")
        rb = pool.tile([128, S], F16, name="rb")
        nc.gpsimd.dma_start(ra[:, :], ropeA[:, :])
        nc.gpsimd.dma_start(rb[:, :], ropeB[:, :])
        ident = pool.tile([128, 128], F32, name="ident")
        make_identity(nc, ident[:, :])
        ones1 = pool.tile([128, 128], F16, name="ones1")
        nc.gpsimd.memset(ones1[:, :], 1.0)

        # ---- big input DMAs ----
        # wk/wq first (small, unblock the PE), then x one block per kb so
        # the head-0 K/Q projections can accumulate into held PSUM tiles as
        # each block lands. One TILE per kb block: the Tile framework tracks
        # dependencies per tile, so a single xsb tile would make the first
        # matmul wait for all 16 DMAs. Issues alternate SP/ACT queues.
        xsb = [pool.tile([128, S], F16, name=f"xsb{kb}") for kb in range(KB)]
        wkt = pool.tile([128, KB, 512], F16, name="wt", tag="w", bufs=2)
        wqt = pool.tile([128, KB, 512], F16, name="wt", tag="w", bufs=2)
        nc.sync.dma_start(wkt[:, :, :], wk_p[:, :, :])
        nc.scalar.dma_start(wqt[:, :, :], wq_p[:, :, :])
        engs = [nc.sync, nc.scalar, nc.gpsimd]
        for kb in range(KB):
            engs[kb % 3].dma_start(xsb[kb][:, :], xT_r[kb])

        # ---- persistent q/k/v head tiles ([head_dim, seq] layout) ----
        qh = [pool.tile([128, S], F16, name=f"qh{h}") for h in range(HPC)]
        kh = [pool.tile([128, S], F16, name=f"kh{h}") for h in range(HPC)]
        vh = [pool.tile([128, S], F16, name=f"vh{h}") for h in range(HPC)]

        # per-head row vectors live at partition 32*h (engine ops only
        # support start partitions that are multiples of 32)
        ds_diag = pool.tile([128, S], F16, name="ds_diag")
        ds_sum = pool.tile([128, S], F16, name="ds_sum")
        w4 = pool.tile([128, S], F16, name="w4")
        # 2 accum columns per sq block (exp is done in 1024-wide halves)
        sumf = [pool.tile([128, 2 * SB], F32, name=f"sumf{h}")
                for h in range(HPC)]

        # fp8 copies of the roped q/k for the score matmuls (DoubleRow
        # packs the 128 head dims as 2x64: partition p holds dh=p and
        # dh=p+64 side by side in the free dim). The scores feed only the
        # softmax normalizer; numpy simulation puts the fp8 cost at
        # ~4e-3 relative on the final output. diag/attn stay fp16.
        f8t = {}

        def cast_fp8(sel, h):
            src = (qh if sel == "q" else kh)[h]
            full = pool.tile([128, S], F8, name="f8full", tag="f8s", bufs=2)
            nc.vector.tensor_copy(full[:, :], src[:, :])
            two = pool.tile([64, 2, S], F8, name="f8two", tag=f"f8{sel}",
                            bufs=2)
            f8t[(sel, h)] = two
            nc.gpsimd.dma_start(two[:, 0, :], full[0:64, :])
            nc.gpsimd.dma_start(two[:, 1, :], full[64:128, :])

        def proj_mms(wt, mt, sc):
            # (wt[:, :, mt] block).T @ x chunk -> a rotating psum tile
            ps = pp.tile([128, 512], F32, name="mmps", tag="mm", bufs=4)
            for kb in range(KB):
                nc.tensor.matmul(
                    ps[:, :],
                    wt[:, kb, mt * 128:(mt + 1) * 128],
                    xsb[kb][:, sc * 512:(sc + 1) * 512],
                    start=(kb == 0), stop=(kb == KB - 1))
            return ps

        def proj_chunk(wt, dests, mt, sc):
            # psum->sbuf copy on DVE (ACT is reserved for the exp stream).
            ps = proj_mms(wt, mt, sc)
            nc.vector.tensor_copy(
                dests[mt][:, sc * 512:(sc + 1) * 512], ps[:, :])

        def rope_half(dst, c):
            # dst half (in place): top = te*cos - to*sin ; bot = te*sin+to*cos
            # ra = [cosT; cosT], rb = [-sinT; sinT]; swap = halves exchanged.
            sl = slice(c * 1024, (c + 1) * 1024)
            # SWDGE (gpsimd) keeps this 1 queue -> 1 sem; a wide HWDGE
            # sbuf->sbuf DMA fans out over many queues and blows the
            # consumer's sync-wait slot budget.
            swp = pool.tile([128, 1024], F16, name="swp", tag="swp", bufs=2)
            nc.gpsimd.dma_start(swp[0:64, :], dst[64:128, sl])
            nc.gpsimd.dma_start(swp[64:128, :], dst[0:64, sl])
            u = pool.tile([128, 1024], F16, name="u", tag="sc", bufs=2)
            nc.vector.tensor_mul(u[:, :], dst[:, sl], ra[:, sl])
            v2 = pool.tile([128, 1024], F16, name="v2", tag="sc", bufs=2)
            nc.vector.tensor_mul(v2[:, :], swp[:, :], rb[:, sl])
            nc.vector.tensor_add(dst[:, sl], u[:, :], v2[:, :])

        def diag_half(h, c):
            # ds_diag[32h, s-half] = sum_m qh[h][m,s] * kh[h][m,s]
            hp = 32 * h
            sl = slice(c * 1024, (c + 1) * 1024)
            pr = pool.tile([128, 1024], F16, name="pr", tag="pr", bufs=1)
            nc.vector.tensor_mul(pr[:, :], qh[h][:, sl], kh[h][:, sl])
            for cc in range(2):
                dps = pp.tile([128, 512], F32, name="mmps", tag="mm", bufs=4)
                nc.tensor.matmul(dps[:, :], ones1[:, :],
                                 pr[:, cc * 512:(cc + 1) * 512],
                                 start=True, stop=True)
                o = (2 * c + cc) * 512
                nc.vector.tensor_copy(ds_diag[hp:hp + 1, o:o + 512],
                                      dps[hp:hp + 1, :])

        # ====== scores stream ======
        ex = pool.tile([128, 1024], F16, name="ex")

        def scores_half(h, sq, half):
            # 2 fp8 DoubleRow score MMs into a rotating [128,1024] psum
            # half, one exp with fused row-sum into sumf[h][:, half*SB+sq].
            sps = pp.tile([128, 1024], F32, name="sps", tag="sco", bufs=2)
            q8 = f8t[("q", h)]
            k8 = f8t[("k", h)]
            for cc in range(2):
                ck = 2 * half + cc
                nc.tensor.matmul(sps[:, cc * 512:(cc + 1) * 512],
                                 q8[:, :, sq * 128:(sq + 1) * 128],
                                 k8[:, :, ck * 512:(ck + 1) * 512],
                                 start=True, stop=True, perf_mode=DR)
            col = half * SB + sq
            nc.scalar.activation(ex[:, :], sps[:, :], AF.Exp,
                                 accum_out=sumf[h][:, col:col + 1])

        rsq = {}

        def head_sum_pre(h):
            # DVE-only piece: sum the 2 half-accums, reciprocal
            stot = pool.tile([128, SB], F32, name="stot", tag="rs", bufs=2)
            nc.vector.tensor_add(stot[:, :], sumf[h][:, 0:SB],
                                 sumf[h][:, SB:2 * SB])
            rsq[h] = pool.tile([128, SB], F32, name="rs", tag="rs", bufs=2)
            nc.vector.reciprocal(rsq[h][:, :], stot[:, :])

        def head_sum_post(h):
            # transpose -> [1,S] ds_sum row (PE piece, emitted after a
            # filler so the PE queue has work while the DVE piece resolves)
            hp = 32 * h
            tps = pp.tile([16, 128], F32, name="mmps", tag="mm", bufs=4)
            nc.tensor.transpose(tps[:, :], rsq[h][:, :], ident[:, :])
            st = pool.tile([16, 128], F16, name="st", tag="st", bufs=2)
            nc.vector.tensor_copy(st[:, :], tps[:, :])
            nc.gpsimd.dma_start(ds_sum[hp:hp + 1, :], st[:, :])

        def pair_head(p, units):
            # w = exp(diag) * recip(sumexp); attn = w (bcast) * v, into kh.
            # ACT exps first (no PE coupling), then broadcast+scale chunk
            # groups interleaved with independent PE units.
            for h in (2 * p, 2 * p + 1):
                hp = 32 * h
                nc.scalar.activation(w4[hp:hp + 1, :],
                                     ds_diag[hp:hp + 1, :], AF.Exp)
            if units:
                units.pop(0)()
            for h in (2 * p, 2 * p + 1):
                hp = 32 * h
                nc.vector.tensor_mul(w4[hp:hp + 1, :], w4[hp:hp + 1, :],
                                     ds_sum[hp:hp + 1, :])
            for ck in range(SC):
                for h in (2 * p, 2 * p + 1):
                    hp = 32 * h
                    # K=1 outer-product broadcast of the w row to 128 parts
                    bps = pp.tile([128, 512], F32, name="mmps", tag="mm",
                                  bufs=4)
                    nc.tensor.matmul(bps[:, :], ones1[hp:hp + 1, :],
                                     w4[hp:hp + 1, ck * 512:(ck + 1) * 512],
                                     start=True, stop=True,
                                     tile_position=(hp, 0))
                    # attn scaling straight from psum (no bounce buffer)
                    nc.vector.tensor_mul(kh[h][:, ck * 512:(ck + 1) * 512],
                                         bps[:, :],
                                         vh[h][:, ck * 512:(ck + 1) * 512])
                for _ in range(2):
                    if units:
                        units.pop(0)()

        def oproj_unit(p, sb, ncx, yts, copy_eng):
            # one 128-row x 512-col chunk of the pair-p output projection
            h0, h1 = 2 * p, 2 * p + 1
            ps = pp.tile([128, 512], F32, name="mmps", tag="mm", bufs=4)
            for i, h in enumerate((h0, h1)):
                nc.tensor.matmul(
                    ps[:, :], kh[h][:, sb * 128:(sb + 1) * 128],
                    wot[:, h, ncx * 512:(ncx + 1) * 512],
                    start=(i == 0), stop=(i == 1))
            dst = yts[:, ncx * 512:(ncx + 1) * 512]
            if copy_eng == "act":
                nc.scalar.activation(dst, ps[:, :], AF.Copy)
            else:
                nc.vector.tensor_copy(dst, ps[:, :])
            if ncx == SC - 1:
                nc.sync.dma_start(y[p, sb * 128:(sb + 1) * 128, :],
                                  yts[:, :])

        # ================= emission =================
        # Phase 0: head-0 K and Q projections in kb-major order - one
        # accumulation step into 6 held PSUM tiles per x block as it lands,
        # so the PE tracks the x DMA stream instead of waiting for all of x.
        kA = pp.tile([128, 1024], F32, name="sps", tag="sco", bufs=2)
        kB = pp.tile([128, 1024], F32, name="sps", tag="sco", bufs=2)
        q4 = [pp.tile([128, 512], F32, name="mmps", tag="mm", bufs=4)
              for _ in range(4)]
        for kb in range(KB):
            st_, sp_ = (kb == 0), (kb == KB - 1)
            for sc in range(2):
                nc.tensor.matmul(kA[:, sc * 512:(sc + 1) * 512],
                                 wkt[:, kb, 0:128],
                                 xsb[kb][:, sc * 512:(sc + 1) * 512],
                                 start=st_, stop=sp_)
            for sc in range(2, 4):
                nc.tensor.matmul(kB[:, (sc - 2) * 512:(sc - 1) * 512],
                                 wkt[:, kb, 0:128],
                                 xsb[kb][:, sc * 512:(sc + 1) * 512],
                                 start=st_, stop=sp_)
            for sc in range(4):
                nc.tensor.matmul(q4[sc][:, :], wqt[:, kb, 0:128],
                                 xsb[kb][:, sc * 512:(sc + 1) * 512],
                                 start=st_, stop=sp_)
        # drain + rope, interleaved with head-1 K projection chunks so the
        # PE has queued work while the DVE/gpsimd rope chain resolves. Q0's
        # mm-pool psums are drained before the K1 chunks rotate onto their
        # banks; the K1 copies are emitted AFTER the rope ops (DVE is
        # in-order, so the reverse would head-of-line block the ropes).
        nc.vector.tensor_copy(kh[0][:, 0:1024], kA[:, :])
        nc.vector.tensor_copy(qh[0][:, 0:512], q4[0][:, :])
        nc.vector.tensor_copy(qh[0][:, 512:1024], q4[1][:, :])
        p0 = proj_mms(wkt, 1, 0)
        rope_half(kh[0], 0)
        nc.vector.tensor_copy(kh[1][:, 0:512], p0[:, :])
        nc.vector.tensor_copy(kh[0][:, 1024:2048], kB[:, :])
        nc.vector.tensor_copy(qh[0][:, 1024:1536], q4[2][:, :])
        nc.vector.tensor_copy(qh[0][:, 1536:2048], q4[3][:, :])
        p1 = proj_mms(wkt, 1, 1)
        rope_half(kh[0], 1)
        nc.vector.tensor_copy(kh[1][:, 512:1024], p1[:, :])
        p2 = proj_mms(wkt, 1, 2)
        rope_half(qh[0], 0)
        nc.vector.tensor_copy(kh[1][:, 1024:1536], p2[:, :])
        p3 = proj_mms(wkt, 1, 3)
        rope_half(qh[0], 1)
        nc.vector.tensor_copy(kh[1][:, 1536:2048], p3[:, :])
        cast_fp8("q", 0)
        cast_fp8("k", 0)

        # Filler micro-units (~1.7us of PE each), emitted between score
        # matmul groups. Small units distribute evenly into the ~2us of PE
        # slack per score block; a monolithic 3.5us chunk can't. Each proj
        # chunk is two halves sharing one psum tile (held across the gap);
        # ropes/diags are standalone units. Order respects cross-engine
        # in-order queues (an instruction emitted before its producer
        # would head-of-line block its engine).
        def chunk_units(wt_f, dests, mt, sc):
            cell = []

            def fa():
                ps = pp.tile([128, 512], F32, name="mmps", tag="mm", bufs=4)
                cell.append(ps)
                for kb in range(KB // 2):
                    nc.tensor.matmul(
                        ps[:, :], wt_f()[:, kb, mt * 128:(mt + 1) * 128],
                        xsb[kb][:, sc * 512:(sc + 1) * 512],
                        start=(kb == 0), stop=False)

            def fb():
                ps = cell[0]
                for kb in range(KB // 2, KB):
                    nc.tensor.matmul(
                        ps[:, :], wt_f()[:, kb, mt * 128:(mt + 1) * 128],
                        xsb[kb][:, sc * 512:(sc + 1) * 512],
                        start=False, stop=(kb == KB - 1))
                nc.vector.tensor_copy(
                    dests[mt][:, sc * 512:(sc + 1) * 512], ps[:, :])
            return [fa, fb]

        def rope_unit(dst, c):
            return [lambda: rope_half(dst, c)]

        def diag_units(h):
            return [lambda: diag_half(h, 0), lambda: diag_half(h, 1)]

        wvt = None
        wot = None

        def load_wv():
            nonlocal wvt
            wvt = pool.tile([128, KB, 512], F16, name="wt", tag="w", bufs=2)
            nc.sync.dma_start(wvt[:, :, :], wv_p[:, :, :])

        def load_wo():
            nonlocal wot
            wot = pool.tile([128, HPC, S], F16, name="wt", tag="w", bufs=2)
            nc.sync.dma_start(wot[:, :, :], wo_p[:, :, :])

        def head_units(mt):
            # micro-units for one head's K then Q projections, with rope
            # halves as soon as their chunks land and diags after the ropes
            units = []
            for sc in range(SC):
                units += chunk_units(lambda: wkt, kh, mt, sc)
                if sc == 1:
                    units += rope_unit(kh[mt], 0)
                if sc == 3:
                    units += rope_unit(kh[mt], 1)
            for sc in range(SC):
                units += chunk_units(lambda: wqt, qh, mt, sc)
                if sc == 1:
                    units += rope_unit(qh[mt], 0)
                    units.append(lambda mt=mt: diag_half(mt, 0))
                if sc == 3:
                    units += rope_unit(qh[mt], 1)
                    units.append(lambda mt=mt: diag_half(mt, 1))
            return units

        def v_units(mt):
            units = []
            for sc in range(SC):
                units += chunk_units(lambda: wvt, vh, mt, sc)
            return units

        def k_units(mt):
            units = []
            for sc in range(SC):
                units += chunk_units(lambda: wkt, kh, mt, sc)
                if sc == 1:
                    units += rope_unit(kh[mt], 0)
                if sc == 3:
                    units += rope_unit(kh[mt], 1)
            return units

        def q_units(mt):
            units = []
            for sc in range(SC):
                units += chunk_units(lambda: wqt, qh, mt, sc)
                if sc == 1:
                    units += rope_unit(qh[mt], 0)
                    units.append(lambda mt=mt: diag_half(mt, 0))
                if sc == 3:
                    units += rope_unit(qh[mt], 1)
                    units.append(lambda mt=mt: diag_half(mt, 1))
                    units.append(lambda mt=mt: cast_fp8("q", mt))
                    units.append(lambda mt=mt: cast_fp8("k", mt))
            return units

        # one flat micro-unit list consumed across the h0+h1 streams with
        # proportional pacing (avoids fat early slots + starved late slots)
        fill_a = ([lambda: diag_half(0, 0), lambda: diag_half(0, 1)]
                  + rope_unit(kh[1], 0) + rope_unit(kh[1], 1)
                  + q_units(1) + k_units(2) + q_units(2) + k_units(3)
                  + [load_wv] + q_units(3) + [load_wo]
                  + v_units(0) + v_units(1))

        yts = {}
        ofill = [(0, sb, ncx) for sb in range(SB) for ncx in range(SC)]

        def oproj_pop(n, copy_eng="dve", keep=0):
            for _ in range(n):
                if len(ofill) <= keep:
                    return
                p, sb, ncx = ofill.pop(0)
                if ncx == 0:
                    yts[p] = pool.tile([128, S], F16, name="yt",
                                       tag="yt", bufs=2)
                oproj_unit(p, sb, ncx, yts[p], copy_eng)

        def stream(h, fill, frac=1.0, per_sq_oproj=0):
            # scores BEFORE the slot's fillers: Bacc lowers cross-engine
            # deps as monotonic queue-count gates, so an exp emitted after
            # a filler would wait for that filler's DVE copy too.
            take = int(round(len(fill) * frac))
            taken = 0
            for sq in range(SB):
                scores_half(h, sq, 0)
                scores_half(h, sq, 1)
                tgt = take * (sq + 1) // SB
                while taken < tgt and fill:
                    fill.pop(0)()
                    taken += 1
                if per_sq_oproj:
                    # hold back 5 pair-0 units to feed the PE through the
                    # pair-1 boundary chain
                    oproj_pop(per_sq_oproj, keep=5)
            head_sum_pre(h)

        # the last V1 units migrate into the pair-0 boundary (PE work that
        # covers the hst/pair serial chain AND thins the h1 stream)
        vb = fill_a[-4:]
        del fill_a[-4:]

        stream(0, fill_a, frac=0.5)
        fill_a.pop(0)()
        head_sum_post(0)
        stream(1, fill_a)
        fill_b = v_units(2) + v_units(3)
        vb += [fill_b.pop(0) for _ in range(5)]
        head_sum_post(1)
        pair_head(0, vb)
        for f in vb:
            f()
        stream(2, fill_b, per_sq_oproj=2)
        oproj_pop(1, keep=8)
        head_sum_post(2)
        stream(3, [], per_sq_oproj=2)

        def reserve_unit():
            def f():
                oproj_pop(1)
            return f

        head_sum_post(3)
        pair_head(1, [reserve_unit() for _ in range(8)])

        # tail: pair-1 output projection; psum->sbuf copies alternate
        # DVE/ACT (ACT is idle by now).
        ofill += [(1, sb, ncx) for sb in range(SB) for ncx in range(SC)]
        i = 0
        while ofill:
            oproj_pop(1, "act" if i % 2 else "dve")
            i += 1

    nc.compile()
    return nc


def _get_nc():
    if "nc" not in _CACHE:
        _CACHE["nc"] = _build_nc()
    return _CACHE["nc"]


_PERM = np.concatenate([np.arange(0, DH, 2), np.arange(1, DH, 2)])


def _host_inputs(x, rope_cos, rope_sin, Wq, Wk, Wv, Wo):
    """Build the 8 per-core input maps."""
    f16 = np.float16
    cosT = np.ascontiguousarray(np.asarray(rope_cos, np.float32)[0, :, 0, :].T)
    sinT = np.ascontiguousarray(np.asarray(rope_sin, np.float32)[0, :, 0, :].T)
    ra = np.concatenate([cosT, cosT], 0).astype(f16)
    rb = np.concatenate([-sinT, sinT], 0).astype(f16)

    Wq = np.asarray(Wq, np.float32)
    Wk = np.asarray(Wk, np.float32)
    Wv = np.asarray(Wv, np.float32)
    Wo = np.asarray(Wo, np.float32)
    x = np.asarray(x, np.float32)

    xTb = [np.ascontiguousarray(x[b].T).astype(f16) for b in range(B)]
    scale = DH ** -0.5

    def pm(arr, nblk):
        # partition-major DMA layout: [p, blk*inner + m] = arr[blk*128+p, m]
        inner = arr.shape[1]
        return np.ascontiguousarray(
            arr.reshape(nblk, 128, inner).transpose(1, 0, 2)
            .reshape(128, nblk * inner))

    in_maps = []
    for core in range(NCORES):
        b, g = divmod(core, HPC)
        hs = g * HPC
        rows = np.concatenate(
            [h * DH + _PERM for h in range(hs, hs + HPC)])      # deinterleave
        rows_v = np.arange(hs * DH, (hs + HPC) * DH)
        in_maps.append({
            "xT": xTb[b],
            "wq": pm((Wq[rows] * scale).T, KB).astype(f16),
            "wk": pm(Wk[rows].T, KB).astype(f16),
            "wv": pm(Wv[rows_v].T, KB).astype(f16),
            "wo": pm(Wo[:, rows_v].T, HPC).astype(f16),
            "ropeA": ra,
            "ropeB": rb,
        })
    return in_maps


def kernel(x, rope_cos, rope_sin, Wq, Wk, Wv, Wo, _trace=False, _trace_cores=None):
    from concourse.bass_utils import run_bass_kernel_spmd

    nc = _get_nc()
    in_maps = _host_inputs(x, rope_cos, rope_sin, Wq, Wk, Wv, Wo)
    res = run_bass_kernel_spmd(nc, in_maps, list(range(NCORES)),
                               trace=_trace, trace_cores=_trace_cores)
    _CACHE["last_result"] = res

    out = np.zeros((B, S, D), np.float32)
    for core in range(NCORES):
        b = core // HPC
        out[b] += res.results[core]["y"].astype(np.float32).sum(axis=0)
    return out


# revision 47
# speedup vs baseline: 1.2483x; 1.2483x over previous
"""Trainium2 Bass kernel for the MHA-with-diagonal-softmax module.

Computation (per batch b):
    q = rope(x @ Wq.T), k = rope(x @ Wk.T), v = x @ Wv.T      (per head, DH=128)
    sumexp[s,h] = sum_k exp(q_h[s] . k_h[k] * DH^-0.5)
    diag[s,h]   = q_h[s] . k_h[s] * DH^-0.5
    w = exp(diag) / sumexp
    out = (w * v) @ Wo.T

Sharding: 8 cores = 2 (batch) x 4 (head groups of 4 heads).
Each core computes q/k/v for its 4 heads in transposed [head_dim, seq]
layout, the per-position softmax-diagonal weights, and a partial output
projection (its heads' rows of Wo), written as 2 head-pair partials that
the host sums.

Schedule (v2): the exp(scores) stream on the ACT engine is the second-
largest engine load (~180us) and is started as early as possible (~32us,
right after head 0's K/Q projections + rope). All remaining PE work
(K/Q proj heads 1-3, V proj, output proj of pair 0) is emitted as
fine-grained filler between score matmuls so the PE queue never stalls
(in-order queues: a waiting instruction blocks everything behind it) and
the PE stays at the 2.4GHz p-state. Scores PSUM is a rotating 3-buffer
of [128,1024] halves (6 banks) + a 2-buffer [128,512] pool (2 banks) for
everything else, so score matmuls never wait on exp drain.

On-chip dtype is fp16 (same PE throughput as bf16, 8x lower rounding
error - matters because exp() amplifies absolute score error), with fp32
PSUM accumulation everywhere.
"""

import numpy as np
from contextlib import ExitStack

# Problem constants (hardcoded per harness contract).
B, S, D, H, DH = 2, 2048, 2048, 16, 128
HPC = 4            # heads per core
NHL = HPC * DH     # 512 local head dims per core
KB = D // 128      # 16 contraction blocks
SB = S // 128      # 16 seq blocks of 128
SC = S // 512      # 4 seq/emb chunks of 512
NCORES = 8

_CACHE = {}


def _build_nc():
    import concourse.bass as bass
    import concourse.tile as tile
    from concourse import bacc, mybir
    from concourse.masks import make_identity

    F16 = mybir.dt.float16
    F32 = mybir.dt.float32
    F8 = mybir.dt.float8e4
    AF = mybir.ActivationFunctionType
    DR = mybir.MatmulPerfMode.DoubleRow

    # Bacc (not raw Bass): its compile() splits multi-sem waits into
    # event-semaphore instructions - HW allows at most 1 wait per inst.
    nc = bacc.Bacc("TRN2", target_bir_lowering=False, debug=False)

    # weights arrive pre-arranged partition-major on the host so each DMA
    # is 128 x 8KB contiguous descriptors (1KB-row descriptors measured
    # ~120GB/s and hog the 4-deep DMA rings)
    xT = nc.dram_tensor("xT", [D, S], F16, kind="ExternalInput").ap()
    wq = nc.dram_tensor("wq", [128, KB * 512], F16, kind="ExternalInput").ap()
    wk = nc.dram_tensor("wk", [128, KB * 512], F16, kind="ExternalInput").ap()
    wv = nc.dram_tensor("wv", [128, KB * 512], F16, kind="ExternalInput").ap()
    wo = nc.dram_tensor("wo", [128, HPC * S], F16, kind="ExternalInput").ap()
    ropeA = nc.dram_tensor("ropeA", [128, S], F16, kind="ExternalInput").ap()
    ropeB = nc.dram_tensor("ropeB", [128, S], F16, kind="ExternalInput").ap()
    y = nc.dram_tensor("y", [2, S, D], F16, kind="ExternalOutput").ap()

    xT_r = xT.rearrange("(a p) s -> a p s", p=128)
    wq_p = wq.rearrange("p (a m) -> p a m", a=KB)
    wk_p = wk.rearrange("p (a m) -> p a m", a=KB)
    wv_p = wv.rearrange("p (a m) -> p a m", a=KB)
    wo_p = wo.rearrange("p (h n) -> p h n", h=HPC)

    with tile.TileContext(nc) as tc, ExitStack() as ctx:
        pool = ctx.enter_context(tc.tile_pool(name="sb", bufs=1))
        pp = ctx.enter_context(tc.tile_pool(name="ps", bufs=1, space="PSUM"))

        # ---- constants (gpsimd SWDGE: small, keeps HWDGE queues for x/w) --
        ra = pool.tile([128, S], F16, name="ra")
        rb = pool.tile([128, S], F16, name="rb")
        nc.gpsimd.dma_start(ra[:, :], ropeA[:, :])
        nc.gpsimd.dma_start(rb[:, :], ropeB[:, :])
        ident = pool.tile([128, 128], F32, name="ident")
        make_identity(nc, ident[:, :])
        ones1 = pool.tile([128, 128], F16, name="ones1")
        nc.gpsimd.memset(ones1[:, :], 1.0)

        # ---- big input DMAs ----
        # wk/wq first (small, unblock the PE), then x one block per kb so
        # the head-0 K/Q projections can accumulate into held PSUM tiles as
        # each block lands. One TILE per kb block: the Tile framework tracks
        # dependencies per tile, so a single xsb tile would make the first
        # matmul wait for all 16 DMAs. Issues alternate SP/ACT queues.
        xsb = [pool.tile([128, S], F16, name=f"xsb{kb}") for kb in range(KB)]
        wkt = pool.tile([128, KB, 512], F16, name="wt", tag="w", bufs=2)
        wqt = pool.tile([128, KB, 512], F16, name="wt", tag="w", bufs=2)
        nc.sync.dma_start(wkt[:, :, :], wk_p[:, :, :])
        nc.scalar.dma_start(wqt[:, :, :], wq_p[:, :, :])
        engs = [nc.sync, nc.scalar, nc.gpsimd]
        for kb in range(KB):
            engs[kb % 3].dma_start(xsb[kb][:, :], xT_r[kb])

        # ---- persistent q/k/v head tiles ([head_dim, seq] layout) ----
        qh = [pool.tile([128, S], F16, name=f"qh{h}") for h in range(HPC)]
        kh = [pool.tile([128, S], F16, name=f"kh{h}") for h in range(HPC)]
        vh = [pool.tile([128, S], F16, name=f"vh{h}") for h in range(HPC)]

        # per-head row vectors live at partition 32*h (engine ops only
        # support start partitions that are multiples of 32)
        ds_diag = pool.tile([128, S], F16, name="ds_diag")
        ds_sum = pool.tile([128, S], F16, name="ds_sum")
        w4 = pool.tile([128, S], F16, name="w4")
        # 2 accum columns per sq block (exp is done in 1024-wide halves)
        sumf = [pool.tile([128, 2 * SB], F32, name=f"sumf{h}")
                for h in range(HPC)]

        # fp8 copies of the roped q/k for the score matmuls (DoubleRow
        # packs the 128 head dims as 2x64: partition p holds dh=p and
        # dh=p+64 side by side in the free dim). The scores feed only the
        # softmax normalizer; numpy simulation puts the fp8 cost at
        # ~4e-3 relative on the final output. diag/attn stay fp16.
        f8t = {}

        def cast_fp8(sel, h):
            src = (qh if sel == "q" else kh)[h]
            full = pool.tile([128, S], F8, name="f8full", tag="f8s", bufs=2)
            nc.vector.tensor_copy(full[:, :], src[:, :])
            two = pool.tile([64, 2, S], F8, name="f8two", tag=f"f8{sel}",
                            bufs=2)
            f8t[(sel, h)] = two
            nc.gpsimd.dma_start(two[:, 0, :], full[0:64, :])
            nc.gpsimd.dma_start(two[:, 1, :], full[64:128, :])

        def proj_mms(wt, mt, sc):
            # (wt[:, :, mt] block).T @ x chunk -> a rotating psum tile
            ps = pp.tile([128, 512], F32, name="mmps", tag="mm", bufs=4)
            for kb in range(KB):
                nc.tensor.matmul(
                    ps[:, :],
                    wt[:, kb, mt * 128:(mt + 1) * 128],
                    xsb[kb][:, sc * 512:(sc + 1) * 512],
                    start=(kb == 0), stop=(kb == KB - 1))
            return ps

        def proj_chunk(wt, dests, mt, sc):
            # psum->sbuf copy on DVE (ACT is reserved for the exp stream).
            ps = proj_mms(wt, mt, sc)
            nc.vector.tensor_copy(
                dests[mt][:, sc * 512:(sc + 1) * 512], ps[:, :])

        def rope_half(dst, c):
            # dst half (in place): top = te*cos - to*sin ; bot = te*sin+to*cos
            # ra = [cosT; cosT], rb = [-sinT; sinT]; swap = halves exchanged.
            sl = slice(c * 1024, (c + 1) * 1024)
            # SWDGE (gpsimd) keeps this 1 queue -> 1 sem; a wide HWDGE
            # sbuf->sbuf DMA fans out over many queues and blows the
            # consumer's sync-wait slot budget.
            swp = pool.tile([128, 1024], F16, name="swp", tag="swp", bufs=2)
            nc.gpsimd.dma_start(swp[0:64, :], dst[64:128, sl])
            nc.gpsimd.dma_start(swp[64:128, :], dst[0:64, sl])
            u = pool.tile([128, 1024], F16, name="u", tag="sc", bufs=2)
            nc.vector.tensor_mul(u[:, :], dst[:, sl], ra[:, sl])
            v2 = pool.tile([128, 1024], F16, name="v2", tag="sc", bufs=2)
            nc.vector.tensor_mul(v2[:, :], swp[:, :], rb[:, sl])
            nc.vector.tensor_add(dst[:, sl], u[:, :], v2[:, :])

        def diag_half(h, c):
            # ds_diag[32h, s-half] = sum_m qh[h][m,s] * kh[h][m,s]
            hp = 32 * h
            sl = slice(c * 1024, (c + 1) * 1024)
            pr = pool.tile([128, 1024], F16, name="pr", tag="pr", bufs=2)
            nc.vector.tensor_mul(pr[:, :], qh[h][:, sl], kh[h][:, sl])
            for cc in range(2):
                dps = pp.tile([128, 512], F32, name="mmps", tag="mm", bufs=4)
                nc.tensor.matmul(dps[:, :], ones1[:, :],
                                 pr[:, cc * 512:(cc + 1) * 512],
                                 start=True, stop=True)
                o = (2 * c + cc) * 512
                nc.vector.tensor_copy(ds_diag[hp:hp + 1, o:o + 512],
                                      dps[hp:hp + 1, :])

        # ====== scores stream ======
        ex = pool.tile([128, 1024], F16, name="ex")

        def scores_half(h, sq, half):
            # 2 score MMs into a rotating [128,1024] psum half (matmul
            # output must fit one PSUM bank = 512 fp32), one exp with
            # fused row-sum into sumf[h][:, half*SB+sq].
            sps = pp.tile([128, 1024], F32, name="sps", tag="sco", bufs=2)
            for cc in range(2):
                ck = 2 * half + cc
                nc.tensor.matmul(sps[:, cc * 512:(cc + 1) * 512],
                                 qh[h][:, sq * 128:(sq + 1) * 128],
                                 kh[h][:, ck * 512:(ck + 1) * 512],
                                 start=True, stop=True)
            col = half * SB + sq
            nc.scalar.activation(ex[:, :], sps[:, :], AF.Exp,
                                 accum_out=sumf[h][:, col:col + 1])

        rsq = {}

        def head_sum_pre(h):
            # DVE-only piece: sum the 2 half-accums, reciprocal
            stot = pool.tile([128, SB], F32, name="stot", tag="rs", bufs=2)
            nc.vector.tensor_add(stot[:, :], sumf[h][:, 0:SB],
                                 sumf[h][:, SB:2 * SB])
            rsq[h] = pool.tile([128, SB], F32, name="rs", tag="rs", bufs=2)
            nc.vector.reciprocal(rsq[h][:, :], stot[:, :])

        def head_sum_post(h):
            # transpose -> [1,S] ds_sum row (PE piece, emitted after a
            # filler so the PE queue has work while the DVE piece resolves)
            hp = 32 * h
            tps = pp.tile([16, 128], F32, name="mmps", tag="mm", bufs=4)
            nc.tensor.transpose(tps[:, :], rsq[h][:, :], ident[:, :])
            st = pool.tile([16, 128], F16, name="st", tag="st", bufs=2)
            nc.vector.tensor_copy(st[:, :], tps[:, :])
            nc.gpsimd.dma_start(ds_sum[hp:hp + 1, :], st[:, :])

        def pair_head(p, units):
            # w = exp(diag) * recip(sumexp); attn = w (bcast) * v, into kh.
            # ACT exps first (no PE coupling), then broadcast+scale chunk
            # groups interleaved with independent PE units.
            for h in (2 * p, 2 * p + 1):
                hp = 32 * h
                nc.scalar.activation(w4[hp:hp + 1, :],
                                     ds_diag[hp:hp + 1, :], AF.Exp)
            if units:
                units.pop(0)()
            for h in (2 * p, 2 * p + 1):
                hp = 32 * h
                nc.vector.tensor_mul(w4[hp:hp + 1, :], w4[hp:hp + 1, :],
                                     ds_sum[hp:hp + 1, :])
            for ck in range(SC):
                for h in (2 * p, 2 * p + 1):
                    hp = 32 * h
                    # K=1 outer-product broadcast of the w row to 128 parts
                    bps = pp.tile([128, 512], F32, name="mmps", tag="mm",
                                  bufs=4)
                    nc.tensor.matmul(bps[:, :], ones1[hp:hp + 1, :],
                                     w4[hp:hp + 1, ck * 512:(ck + 1) * 512],
                                     start=True, stop=True,
                                     tile_position=(hp, 0))
                    # attn scaling straight from psum (no bounce buffer)
                    nc.vector.tensor_mul(kh[h][:, ck * 512:(ck + 1) * 512],
                                         bps[:, :],
                                         vh[h][:, ck * 512:(ck + 1) * 512])
                for _ in range(2):
                    if units:
                        units.pop(0)()

        def oproj_unit(p, sb, ncx, yts, copy_eng):
            # one 128-row x 512-col chunk of the pair-p output projection
            h0, h1 = 2 * p, 2 * p + 1
            ps = pp.tile([128, 512], F32, name="mmps", tag="mm", bufs=4)
            for i, h in enumerate((h0, h1)):
                nc.tensor.matmul(
                    ps[:, :], kh[h][:, sb * 128:(sb + 1) * 128],
                    wot[:, h, ncx * 512:(ncx + 1) * 512],
                    start=(i == 0), stop=(i == 1))
            dst = yts[:, ncx * 512:(ncx + 1) * 512]
            if copy_eng == "act":
                nc.scalar.activation(dst, ps[:, :], AF.Copy)
            else:
                nc.vector.tensor_copy(dst, ps[:, :])
            if ncx == SC - 1:
                nc.sync.dma_start(y[p, sb * 128:(sb + 1) * 128, :],
                                  yts[:, :])

        # ================= emission =================
        # Phase 0: head-0 K and Q projections in kb-major order - one
        # accumulation step into 6 held PSUM tiles per x block as it lands,
        # so the PE tracks the x DMA stream instead of waiting for all of x.
        kA = pp.tile([128, 1024], F32, name="sps", tag="sco", bufs=2)
        kB = pp.tile([128, 1024], F32, name="sps", tag="sco", bufs=2)
        q4 = [pp.tile([128, 512], F32, name="mmps", tag="mm", bufs=4)
              for _ in range(4)]
        for kb in range(KB):
            st_, sp_ = (kb == 0), (kb == KB - 1)
            for sc in range(2):
                nc.tensor.matmul(kA[:, sc * 512:(sc + 1) * 512],
                                 wkt[:, kb, 0:128],
                                 xsb[kb][:, sc * 512:(sc + 1) * 512],
                                 start=st_, stop=sp_)
            for sc in range(2, 4):
                nc.tensor.matmul(kB[:, (sc - 2) * 512:(sc - 1) * 512],
                                 wkt[:, kb, 0:128],
                                 xsb[kb][:, sc * 512:(sc + 1) * 512],
                                 start=st_, stop=sp_)
            for sc in range(4):
                nc.tensor.matmul(q4[sc][:, :], wqt[:, kb, 0:128],
                                 xsb[kb][:, sc * 512:(sc + 1) * 512],
                                 start=st_, stop=sp_)
        # drain + rope, interleaved with head-1 K projection chunks so the
        # PE has queued work while the DVE/gpsimd rope chain resolves. Q0's
        # mm-pool psums are drained before the K1 chunks rotate onto their
        # banks; the K1 copies are emitted AFTER the rope ops (DVE is
        # in-order, so the reverse would head-of-line block the ropes).
        nc.vector.tensor_copy(kh[0][:, 0:1024], kA[:, :])
        nc.vector.tensor_copy(qh[0][:, 0:512], q4[0][:, :])
        nc.vector.tensor_copy(qh[0][:, 512:1024], q4[1][:, :])
        p0 = proj_mms(wkt, 1, 0)
        rope_half(kh[0], 0)
        nc.vector.tensor_copy(kh[1][:, 0:512], p0[:, :])
        nc.vector.tensor_copy(kh[0][:, 1024:2048], kB[:, :])
        nc.vector.tensor_copy(qh[0][:, 1024:1536], q4[2][:, :])
        nc.vector.tensor_copy(qh[0][:, 1536:2048], q4[3][:, :])
        p1 = proj_mms(wkt, 1, 1)
        rope_half(kh[0], 1)
        nc.vector.tensor_copy(kh[1][:, 512:1024], p1[:, :])
        p2 = proj_mms(wkt, 1, 2)
        rope_half(qh[0], 0)
        nc.vector.tensor_copy(kh[1][:, 1024:1536], p2[:, :])
        p3 = proj_mms(wkt, 1, 3)
        rope_half(qh[0], 1)
        nc.vector.tensor_copy(kh[1][:, 1536:2048], p3[:, :])


        # Filler micro-units (~1.7us of PE each), emitted between score
        # matmul groups. Small units distribute evenly into the ~2us of PE
        # slack per score block; a monolithic 3.5us chunk can't. Each proj
        # chunk is two halves sharing one psum tile (held across the gap);
        # ropes/diags are standalone units. Order respects cross-engine
        # in-order queues (an instruction emitted before its producer
        # would head-of-line block its engine).
        def chunk_units(wt_f, dests, mt, sc):
            cell = []

            def fa():
                ps = pp.tile([128, 512], F32, name="mmps", tag="mm", bufs=4)
                cell.append(ps)
                for kb in range(KB // 2):
                    nc.tensor.matmul(
                        ps[:, :], wt_f()[:, kb, mt * 128:(mt + 1) * 128],
                        xsb[kb][:, sc * 512:(sc + 1) * 512],
                        start=(kb == 0), stop=False)

            def fb():
                ps = cell[0]
                for kb in range(KB // 2, KB):
                    nc.tensor.matmul(
                        ps[:, :], wt_f()[:, kb, mt * 128:(mt + 1) * 128],
                        xsb[kb][:, sc * 512:(sc + 1) * 512],
                        start=False, stop=(kb == KB - 1))
                nc.vector.tensor_copy(
                    dests[mt][:, sc * 512:(sc + 1) * 512], ps[:, :])
            return [fa, fb]

        def rope_unit(dst, c):
            return [lambda: rope_half(dst, c)]

        def diag_units(h):
            return [lambda: diag_half(h, 0), lambda: diag_half(h, 1)]

        wvt = None
        wot = None

        def load_wv():
            nonlocal wvt
            wvt = pool.tile([128, KB, 512], F16, name="wt", tag="w", bufs=2)
            nc.sync.dma_start(wvt[:, :, :], wv_p[:, :, :])

        def load_wo():
            nonlocal wot
            wot = pool.tile([128, HPC, S], F16, name="wt", tag="w", bufs=2)
            nc.sync.dma_start(wot[:, :, :], wo_p[:, :, :])

        def head_units(mt):
            # micro-units for one head's K then Q projections, with rope
            # halves as soon as their chunks land and diags after the ropes
            units = []
            for sc in range(SC):
                units += chunk_units(lambda: wkt, kh, mt, sc)
                if sc == 1:
                    units += rope_unit(kh[mt], 0)
                if sc == 3:
                    units += rope_unit(kh[mt], 1)
            for sc in range(SC):
                units += chunk_units(lambda: wqt, qh, mt, sc)
                if sc == 1:
                    units += rope_unit(qh[mt], 0)
                    units.append(lambda mt=mt: diag_half(mt, 0))
                if sc == 3:
                    units += rope_unit(qh[mt], 1)
                    units.append(lambda mt=mt: diag_half(mt, 1))
            return units

        def v_units(mt):
            units = []
            for sc in range(SC):
                units += chunk_units(lambda: wvt, vh, mt, sc)
            return units

        def k_units(mt):
            units = []
            for sc in range(SC):
                units += chunk_units(lambda: wkt, kh, mt, sc)
                if sc == 1:
                    units += rope_unit(kh[mt], 0)
                if sc == 3:
                    units += rope_unit(kh[mt], 1)
            return units

        def q_units(mt):
            units = []
            for sc in range(SC):
                units += chunk_units(lambda: wqt, qh, mt, sc)
                if sc == 1:
                    units += rope_unit(qh[mt], 0)
                    units.append(lambda mt=mt: diag_half(mt, 0))
                if sc == 3:
                    units += rope_unit(qh[mt], 1)
                    units.append(lambda mt=mt: diag_half(mt, 1))
            return units

        # one flat micro-unit list consumed across the h0+h1 streams with
        # proportional pacing (avoids fat early slots + starved late slots)
        fill_a = ([lambda: diag_half(0, 0), lambda: diag_half(0, 1)]
                  + rope_unit(kh[1], 0) + rope_unit(kh[1], 1)
                  + q_units(1) + k_units(2) + q_units(2) + k_units(3)
                  + [load_wv] + q_units(3) + [load_wo]
                  + v_units(0) + v_units(1))

        yts = {}
        ofill = [(0, sb, ncx) for sb in range(SB) for ncx in range(SC)]

        def oproj_pop(n, copy_eng="dve", keep=0):
            for _ in range(n):
                if len(ofill) <= keep:
                    return
                p, sb, ncx = ofill.pop(0)
                if ncx == 0:
                    yts[p] = pool.tile([128, S], F16, name="yt",
                                       tag="yt", bufs=2)
                oproj_unit(p, sb, ncx, yts[p], copy_eng)

        def stream(h, fill, frac=1.0, per_sq_oproj=0):
            # scores BEFORE the slot's fillers: Bacc lowers cross-engine
            # deps as monotonic queue-count gates, so an exp emitted after
            # a filler would wait for that filler's DVE copy too.
            take = int(round(len(fill) * frac))
            taken = 0
            for sq in range(SB):
                scores_half(h, sq, 0)
                scores_half(h, sq, 1)
                tgt = take * (sq + 1) // SB
                while taken < tgt and fill:
                    fill.pop(0)()
                    taken += 1
                if per_sq_oproj:
                    # hold back 5 pair-0 units to feed the PE through the
                    # pair-1 boundary chain
                    oproj_pop(per_sq_oproj, keep=5)
            head_sum_pre(h)

        # the last V1 units migrate into the pair-0 boundary (PE work that
        # covers the hst/pair serial chain AND thins the h1 stream)
        vb = fill_a[-4:]
        del fill_a[-4:]

        stream(0, fill_a, frac=0.5)
        fill_a.pop(0)()
        head_sum_post(0)
        stream(1, fill_a)
        fill_b = v_units(2) + v_units(3)
        vb += [fill_b.pop(0) for _ in range(5)]
        head_sum_post(1)
        pair_head(0, vb)
        for f in vb:
            f()
        stream(2, fill_b, per_sq_oproj=2)
        oproj_pop(1, keep=8)
        head_sum_post(2)
        stream(3, [], per_sq_oproj=2)

        def reserve_unit():
            def f():
                oproj_pop(1)
            return f

        head_sum_post(3)
        pair_head(1, [reserve_unit() for _ in range(8)])

        # tail: pair-1 output projection; psum->sbuf copies alternate
        # DVE/ACT (ACT is idle by now).
        ofill += [(1, sb, ncx) for sb in range(SB) for ncx in range(SC)]
        i = 0
        while ofill:
            oproj_pop(1, "act" if i % 2 else "dve")
            i += 1

    nc.compile()
    return nc


def _get_nc():
    if "nc" not in _CACHE:
        _CACHE["nc"] = _build_nc()
    return _CACHE["nc"]


_PERM = np.concatenate([np.arange(0, DH, 2), np.arange(1, DH, 2)])


def _host_inputs(x, rope_cos, rope_sin, Wq, Wk, Wv, Wo):
    """Build the 8 per-core input maps."""
    f16 = np.float16
    cosT = np.ascontiguousarray(np.asarray(rope_cos, np.float32)[0, :, 0, :].T)
    sinT = np.ascontiguousarray(np.asarray(rope_sin, np.float32)[0, :, 0, :].T)
    ra = np.concatenate([cosT, cosT], 0).astype(f16)
    rb = np.concatenate([-sinT, sinT], 0).astype(f16)

    Wq = np.asarray(Wq, np.float32)
    Wk = np.asarray(Wk, np.float32)
    Wv = np.asarray(Wv, np.float32)
    Wo = np.asarray(Wo, np.float32)
    x = np.asarray(x, np.float32)

    xTb = [np.ascontiguousarray(x[b].T).astype(f16) for b in range(B)]
    scale = DH ** -0.5

    def pm(arr, nblk):
        # partition-major DMA layout: [p, blk*inner + m] = arr[blk*128+p, m]
        inner = arr.shape[1]
        return np.ascontiguousarray(
            arr.reshape(nblk, 128, inner).transpose(1, 0, 2)
            .reshape(128, nblk * inner))

    in_maps = []
    for core in range(NCORES):
        b, g = divmod(core, HPC)
        hs = g * HPC
        rows = np.concatenate(
            [h * DH + _PERM for h in range(hs, hs + HPC)])      # deinterleave
        rows_v = np.arange(hs * DH, (hs + HPC) * DH)
        in_maps.append({
            "xT": xTb[b],
            "wq": pm((Wq[rows] * scale).T, KB).astype(f16),
            "wk": pm(Wk[rows].T, KB).astype(f16),
            "wv": pm(Wv[rows_v].T, KB).astype(f16),
            "wo": pm(Wo[:, rows_v].T, HPC).astype(f16),
            "ropeA": ra,
            "ropeB": rb,
        })
    return in_maps


def kernel(x, rope_cos, rope_sin, Wq, Wk, Wv, Wo, _trace=False, _trace_cores=None):
    from concourse.bass_utils import run_bass_kernel_spmd

    nc = _get_nc()
    in_maps = _host_inputs(x, rope_cos, rope_sin, Wq, Wk, Wv, Wo)
    res = run_bass_kernel_spmd(nc, in_maps, list(range(NCORES)),
                               trace=_trace, trace_cores=_trace_cores)
    _CACHE["last_result"] = res

    out = np.zeros((B, S, D), np.float32)
    for core in range(NCORES):
        b = core // HPC
        out[b] += res.results[core]["y"].astype(np.float32).sum(axis=0)
    return out
